# revision 33
# baseline (speedup 1.0000x reference)
"""Tensor-parallel causal multi-head attention (RoPE) on 8 TRN2 NeuronCores.

Sharding: 2-way batch x 4-way heads.  Core c handles batch c//4 and heads
[4*(c%4), 4*(c%4)+4).  Each core computes its 4 heads end-to-end for its
batch and writes a bf16 additive partial of that batch's output; the host
sums the 4 partials per batch.

Kernel layout / dtype choices (all-bf16; fp8 was tested and fails the
2e-2 tolerance through the softmax-score path):
  - Everything runs in bf16: projections, scores (Q^T/K^T stored bf16
    after RoPE in fp32 PSUM), probabilities, AV, output projection, and
    the DMA'd output partial.  bf16 matmuls run at full PE rate with
    cheap (FWL) weight loads, vs fp32r's ~187ns serial LDWEIGHTS.
  - x is streamed once per 512-column s-tile and shared by the Q, K and
    V projections of that tile (one 8MB read total).
  - RoPE's rotate-half is a single DVE stream_shuffle: the head dim is
    permuted across partitions (host-side, applied to wq/wk columns and
    cos/sin rows) so each rotate pair sits 16 partitions apart within a
    32-partition quadrant, which STREAM_SHUFFLE can swap (it only
    permutes within 32-partition quadrants).  The rotate sign flip is
    folded into the host-prepared sin.
  - Scores are computed transposed, S^T[k,q], per 2-k-block PSUM group
    [128, 1024] so one scalar-engine exp instruction covers 2 blocks
    (amortizes the 352-cycle ACT overhead).  Softmax denominators come
    from an all-ones [128,128] stationary matmul accumulated alongside
    O^T in PSUM: same row cost as a 1-column ones matmul, but the sum
    lands pre-broadcast across partitions (no gpsimd broadcast needed)
    and, unlike the 1-column variant, it streams at the full PE rate —
    the narrow stationary was silently adding ~90ns to every
    neighbouring matmul.
  - Attention runs as a flat software pipeline over (q-tile, head,
    k-group) tokens with 2 score groups in flight, crossing head and
    q-tile boundaries; each q-tile's output projection is emitted right
    after its last head normalizes, with the next q-tile's first score
    groups already issued so their exps overlap the out-proj matmuls.
  - x and the weights are host-prearranged into their exact SBUF
    layouts so every DMA is a long contiguous per-partition run.
  - 1/sqrt(D) is folded into wq on the host; no max-subtraction (scores
    are O(1) so exp is safe).  PSUM is budgeted exactly: 2x[128,1024]
    score groups + 2x[128,512] accumulators + 2x[128,512] denominator
    tiles = 8 banks.
"""

import math

import numpy as np
import ml_dtypes

import concourse.bass as bass
import concourse.tile as tile
from concourse import bacc, mybir
from concourse.bass_utils import run_bass_kernel_spmd

B, S, HID = 2, 2048, 2048
H, D = 16, 128
NCORES = 8
BGROUP = 4  # cores per batch
HPC = H // BGROUP  # heads per core (4)
DH = HPC * D  # per-core projection width (512)
NHC = HID // 128  # hid chunks (16)
TQ = 512  # q-tile for attention
NKB = S // 128  # k blocks per sequence (16)
F32 = mybir.dt.float32
BF16 = mybir.dt.bfloat16

# stream_shuffle mask: swap 16-partition halves within each 32-part quadrant
SWAP16 = [(i + 16) % 32 for i in range(32)]

# partition permutation: partition 32*q + i holds head-dim
#   d = 16*q + i          (i < 16,  lower half d in [0, 64))
#   d = 64 + 16*q + i-16  (i >= 16, upper half)
# so the rotate-half partner (d <-> d+64) is 16 partitions away in-quadrant.
PERM = np.zeros(128, dtype=np.int64)
for _q in range(4):
    for _i in range(16):
        PERM[32 * _q + _i] = 16 * _q + _i
        PERM[32 * _q + 16 + _i] = 64 + 16 * _q + _i

LAST_EXEC_TIME_NS = None
_CACHE = {}


def _build_device_program():
    nc = bacc.Bacc(
        "TRN2",
        target_bir_lowering=False,
        debug=False,
        enable_asserts=False,
        num_devices=NCORES,
    )
    # x16 is host-prearranged to SBUF layout [p, st, c, s] so each s-tile
    # load is one contiguous 16KB run per partition (full DMA bandwidth)
    x16 = nc.dram_tensor("x16", [128, S // 512, NHC, 512], BF16, kind="ExternalInput").ap()
    # weights likewise prearranged to their SBUF layouts on the host
    wq16 = nc.dram_tensor("wq16", [128, NHC, DH], BF16, kind="ExternalInput").ap()
    wk16 = nc.dram_tensor("wk16", [128, NHC, DH], BF16, kind="ExternalInput").ap()
    wv16 = nc.dram_tensor("wv16", [128, NHC, DH], BF16, kind="ExternalInput").ap()
    wo16 = nc.dram_tensor("wo16", [128, HPC, HID], BF16, kind="ExternalInput").ap()
    cs16 = nc.dram_tensor("cs16", [D, S], BF16, kind="ExternalInput").ap()
    sn16 = nc.dram_tensor("sn16", [D, S], BF16, kind="ExternalInput").ap()
    out = nc.dram_tensor("out", [S, HID], BF16, kind="ExternalOutput").ap()

    with tile.TileContext(nc) as tc:
        _emit_kernel(tc, x16, wq16, wk16, wv16, wo16, cs16, sn16, out)

    nc.compile()
    return nc


def _emit_kernel(tc, x16, wq16, wk16, wv16, wo16, cs16, sn16, out):
    from contextlib import ExitStack

    nc = tc.nc
    with ExitStack() as ctx:
        x16r = x16  # [128, 4, 16, 512] host-prearranged
        wq16r, wk16r, wv16r, wo16r = wq16, wk16, wv16, wo16

        const = ctx.enter_context(tc.tile_pool(name="const", bufs=1))
        seqp = ctx.enter_context(tc.tile_pool(name="seqp", bufs=1))
        xvp = ctx.enter_context(tc.tile_pool(name="xvp", bufs=2))
        ropep = ctx.enter_context(tc.tile_pool(name="ropep", bufs=4))
        ptp = ctx.enter_context(tc.tile_pool(name="ptp", bufs=5))
        atp = ctx.enter_context(tc.tile_pool(name="atp", bufs=2))
        recp = ctx.enter_context(tc.tile_pool(name="recp", bufs=2))
        obp = ctx.enter_context(tc.tile_pool(name="obp", bufs=6))
        psump = ctx.enter_context(tc.tile_pool(name="psump", bufs=2, space="PSUM"))

        # ---- resident inputs ----
        wq16_sb = const.tile([128, NHC, DH], BF16)
        wk16_sb = const.tile([128, NHC, DH], BF16)
        wv16_sb = const.tile([128, NHC, DH], BF16)
        wo16_sb = const.tile([128, HPC, HID], BF16)
        cs_sb = const.tile([128, S], BF16)
        sn_sb = const.tile([128, S], BF16)
        ones16 = const.tile([128, 128], BF16)

        # load order matters: K-proj consumes wk chunk c at ~0.43*c us, and
        # needs wk (1MB) plus the first x s-tile (2MB) inside its first
        # ~7us — more than one DMA queue delivers.  Interleave wk pieces
        # and part of the st0 x tile on the scalar queue while the sync
        # queue carries the rest of the x tile.
        xv0 = xvp.tile([128, NHC, 512], BF16, tag="xv", name="xv0")
        for c0, c1 in ((0, 1), (1, 2), (2, 4), (6, 8), (10, 12), (14, 16)):
            nc.sync.dma_start(out=xv0[:, c0:c1, :], in_=x16r[:, 0, c0:c1, :])
        wk_pieces = [(0, 1), (1, 2), (2, 4), (4, 8), (8, 12), (12, 16)]
        xv_pieces = [(4, 6), (8, 10), (12, 14)]
        for k, (c0, c1) in enumerate(wk_pieces):
            nc.scalar.dma_start(out=wk16_sb[:, c0:c1, :], in_=wk16r[:, c0:c1, :])
            if k >= 2 and k - 2 < len(xv_pieces):
                x0, x1 = xv_pieces[k - 2]
                nc.scalar.dma_start(out=xv0[:, x0:x1, :], in_=x16r[:, 0, x0:x1, :])
        # first RoPE evac (~7us) only needs the first 512 columns of cos/sin
        nc.scalar.dma_start(out=cs_sb[:, 0:512], in_=cs16[:, 0:512])
        nc.scalar.dma_start(out=sn_sb[:, 0:512], in_=sn16[:, 0:512])
        for j in range(2):
            nc.scalar.dma_start(
                out=wv16_sb[:, j * 8 : j * 8 + 8, :], in_=wv16r[:, j * 8 : j * 8 + 8, :]
            )
        for j in range(2):
            nc.scalar.dma_start(
                out=wq16_sb[:, j * 8 : j * 8 + 8, :], in_=wq16r[:, j * 8 : j * 8 + 8, :]
            )
        # rest of cos/sin isn't read until st1's RoPE (~50us) — load after wq
        nc.scalar.dma_start(out=cs_sb[:, 512:S], in_=cs16[:, 512:S])
        nc.scalar.dma_start(out=sn_sb[:, 512:S], in_=sn16[:, 512:S])
        for j in range(2):
            nc.scalar.dma_start(
                out=wo16_sb[:, j * 2 : j * 2 + 2, :], in_=wo16r[:, j * 2 : j * 2 + 2, :]
            )
        nc.vector.memset(ones16[:], 1.0)

        # per-sequence on-chip tensors
        kt16 = seqp.tile([128, HPC, S], BF16)  # K^T roped (perm'd head dim)
        qt16 = seqp.tile([128, HPC, S], BF16)  # Q^T roped+scaled (perm'd)
        v16 = seqp.tile([128, NKB, DH], BF16)  # V row-blocks [k, kb, h*D+d]

        def rope_evac(ps_slice, dst_slice, ss):
            """dst = ps*cos + shuffle16(ps)*sin_folded, for one [128,512]."""
            sh = ropep.tile([128, 512], F32, tag="sh", name="sh")
            nc.vector.stream_shuffle(sh[:], ps_slice, mask=SWAP16)
            t1 = ropep.tile([128, 512], BF16, tag="t1", name="t1")
            nc.gpsimd.tensor_mul(t1[:], sh[:], sn_sb[:, ss : ss + 512])
            t2 = ropep.tile([128, 512], BF16, tag="t2", name="t2")
            nc.vector.tensor_mul(t2[:], ps_slice, cs_sb[:, ss : ss + 512])
            nc.gpsimd.tensor_add(dst_slice, t2[:], t1[:])

        # ---- projections: one x s-tile feeds K, V and Q ----
        # the next tile's prefetch is emitted after this tile's V section
        # (not at the loop top) so it doesn't steal read bandwidth during
        # st0's cold-start crunch; it still lands a full Q-section early
        xv_next = xv0
        for st in range(S // 512):
            s0 = st * 512
            xv = xv_next
            for w_sb, dst in ((wk16_sb, kt16), (wq16_sb, qt16)):
                for pair in range(2):
                    ps = psump.tile([128, 1024], F32, tag="big", name="pskq")
                    for c in range(NHC):
                        for hh in range(2):
                            h = pair * 2 + hh
                            nc.tensor.matmul(
                                ps[:, hh * 512 : (hh + 1) * 512],
                                lhsT=w_sb[:, c, h * 128 : (h + 1) * 128],
                                rhs=xv[:, c, :],
                                start=(c == 0),
                                stop=(c == NHC - 1),
                                skip_group_check=True,
                            )
                    for hh in range(2):
                        h = pair * 2 + hh
                        rope_evac(
                            ps[:, hh * 512 : (hh + 1) * 512],
                            dst[:, h, s0 : s0 + 512],
                            s0,
                        )
                if w_sb is wk16_sb:
                    # V projection for this s-tile (row-block layout)
                    for kb4 in range(4):
                        kb = st * 4 + kb4
                        pv = psump.tile([128, 512], F32, tag="acc", name="pv")
                        for c in range(NHC):
                            nc.tensor.matmul(
                                pv[:],
                                lhsT=xv[:, c, kb4 * 128 : (kb4 + 1) * 128],
                                rhs=wv16_sb[:, c, :],
                                start=(c == 0),
                                stop=(c == NHC - 1),
                            )
                        # DVE evac: scalar-engine PSUM reads steal ~43% of
                        # concurrent PE matmul bandwidth; DVE costs far less
                        nc.vector.tensor_copy(v16[:, kb, :], pv[:])
                    if st + 1 < S // 512:
                        xv_next = xvp.tile(
                            [128, NHC, 512], BF16, tag="xv", name="xv"
                        )
                        nc.sync.dma_start(
                            out=xv_next[:], in_=x16r[:, st + 1, :, :]
                        )

        # ---- attention + output projection ----
        # Flat software pipeline over (qt, h, group) tokens with LOOK score
        # groups in flight, continuing across head and q-tile boundaries so
        # the scalar-engine exp latency stays hidden.  The out-projection of
        # q-tile qt is emitted right after its last head normalizes, with
        # the next q-tile's first score groups already issued so their exps
        # run on ACT while the PE does the out-projection matmuls.
        LOOK = 2
        ngrp_of = lambda qt: (qt + 1) * 2

        def score_group(qt, h, gi):
            # scores for k-blocks (2*gi, 2*gi+1), exp'd into one bf16
            # tile; moving dim trimmed to the causal region
            q0 = qt * TQ
            pss = psump.tile([128, 1024], F32, tag="big", name="pss")
            offs = []
            for t in range(2):
                kb = gi * 2 + t
                off = max(0, kb * 128 - q0)
                offs.append(off)
                nc.tensor.matmul(
                    pss[:, t * 512 + off : t * 512 + 512],
                    lhsT=kt16[:, h, kb * 128 : (kb + 1) * 128],
                    rhs=qt16[:, h, q0 + off : q0 + TQ],
                    start=True,
                    stop=True,
                    skip_group_check=True,
                )
            lo = offs[0]
            pt = ptp.tile([128, 1024], BF16, tag="pt", name="pt")
            nc.scalar.activation(
                pt[:, lo:1024],
                pss[:, lo:1024],
                func=mybir.ActivationFunctionType.Exp,
            )
            for t in range(2):
                kb = gi * 2 + t
                off = offs[t]
                if kb * 128 + 127 > q0:
                    W = TQ - off
                    nc.gpsimd.affine_select(
                        out=pt[:, t * 512 + off : t * 512 + 512],
                        in_=pt[:, t * 512 + off : t * 512 + 512],
                        pattern=[[1, W]],
                        base=q0 + off - kb * 128,
                        channel_multiplier=-1,
                        compare_op=mybir.AluOpType.is_ge,
                        fill=0.0,
                    )
            return pt, offs

        atqs = {}  # qt -> atq tile
        accs = {}  # (qt, h) -> (pso, psl)

        def flush_one(pend):
            qt, h, gi, pt, offs = pend.pop(0)
            nvis = ngrp_of(qt) * 2
            if (qt, h) not in accs:
                pso = psump.tile([128, TQ], F32, tag="acc", name="pso")
                psl = psump.tile([128, TQ], F32, tag="psl", name="psl")
                accs[(qt, h)] = (pso, psl)
            pso, psl = accs[(qt, h)]
            for t in range(2):
                kb = gi * 2 + t
                off = offs[t]
                nc.tensor.matmul(
                    pso[:, off:TQ],
                    lhsT=v16[:, kb, h * D : (h + 1) * D],
                    rhs=pt[:, t * 512 + off : t * 512 + 512],
                    start=(kb == 0),
                    stop=(kb == nvis - 1),
                    skip_group_check=True,
                )
            for t in range(2):
                # full-width all-ones stationary: same cost as a 1-column
                # one, but the softmax denominator comes out pre-broadcast
                # across all partitions (no gpsimd partition_broadcast)
                kb = gi * 2 + t
                off = offs[t]
                nc.tensor.matmul(
                    psl[:, off:TQ],
                    lhsT=ones16[:],
                    rhs=pt[:, t * 512 + off : t * 512 + 512],
                    start=(kb == 0),
                    stop=(kb == nvis - 1),
                    skip_group_check=True,
                )
            if gi == ngrp_of(qt) - 1:
                rb = recp.tile([128, TQ], F32, tag="rb", name="rb")
                nc.vector.reciprocal_approx_fast(out=rb[:], in_=psl[:])
                nc.vector.tensor_mul(atqs[qt][:, h, :], pso[:], rb[:])
                del accs[(qt, h)]
                if h == HPC - 1:
                    emit_outproj(qt)

        def emit_outproj(qt):
            # partial over local heads; pc tiles rotate through the acc and
            # psl tag slots (4 total) so evacuation latency stays hidden
            q0 = qt * TQ
            # psl frees first (after the reciprocal), acc only after the
            # final normalize mul — so start the rotation on psl
            pc_tags = ["psl", "acc"]
            pcn = 0
            for sb in range(TQ // 128):
                r0 = q0 + sb * 128
                for ep in range(2):
                    pcs = []
                    for eu in range(2):
                        tag = pc_tags[pcn % 2]
                        pcs.append(
                            psump.tile([128, 512], F32, tag=tag, name=f"pc{eu}")
                        )
                        pcn += 1
                    for h in range(HPC):
                        for eu in range(2):
                            et = ep * 2 + eu
                            nc.tensor.matmul(
                                pcs[eu][:],
                                lhsT=atq_c[qt][:, h, sb * 128 : (sb + 1) * 128],
                                rhs=wo16_sb[:, h, et * 512 : (et + 1) * 512],
                                start=(h == 0),
                                stop=(h == HPC - 1),
                            )
                    for eu in range(2):
                        et = ep * 2 + eu
                        ob = obp.tile([128, 512], BF16, tag="ob", name="ob")
                        if qt == S // TQ - 1 and eu == 0:
                            # last q-tile: no exps left, so the scalar
                            # engine is free — split evacs across engines
                            # to shorten the kernel tail
                            nc.scalar.copy(ob[:], pcs[eu][:])
                        else:
                            nc.vector.tensor_copy(ob[:], pcs[eu][:])
                        nc.sync.dma_start(
                            out=out[r0 : r0 + 128, et * 512 : (et + 1) * 512],
                            in_=ob[:],
                        )

        atq_c = atqs  # alias used by emit_outproj
        tokens = [
            (qt, h, gi)
            for qt in range(S // TQ)
            for h in range(HPC)
            for gi in range(ngrp_of(qt))
        ]
        pend = []
        for qt, h, gi in tokens:
            if qt not in atqs:
                atqs[qt] = atp.tile([128, HPC, TQ], BF16, tag="atq", name="atq")
            pend.append((qt, h, gi, *score_group(qt, h, gi)))
            if len(pend) > LOOK:
                flush_one(pend)
        while pend:
            flush_one(pend)


def _host_inputs(hidden_states, cos, sin, wq, wk, wv, wo):
    bf = ml_dtypes.bfloat16
    x = np.asarray(hidden_states, dtype=np.float32)  # [B, S, HID]
    cos = np.asarray(cos, dtype=np.float32)  # [B, S, D]
    sin = np.asarray(sin, dtype=np.float32)
    wq = np.asarray(wq, dtype=np.float32)
    wk = np.asarray(wk, dtype=np.float32)
    wv = np.asarray(wv, dtype=np.float32)
    wo = np.asarray(wo, dtype=np.float32)
    scale = 1.0 / math.sqrt(D)

    in_maps = []
    for c in range(NCORES):
        b = c // BGROUP
        g = c % BGROUP
        sl = slice(g * DH, (g + 1) * DH)
        xT = np.ascontiguousarray(x[b].T)  # [HID, S]
        # per-head partition permutation of the q/k head dim
        rows = np.concatenate([h * D + PERM for h in range(HPC)])
        wq_sl = wq[sl][rows] * scale  # [DH, HID], rows permuted per head
        wk_sl = wk[sl][rows]
        csT = cos[b].T[PERM]  # [D, S] permuted
        snT = sin[b].T[PERM]
        neg = PERM < 64  # fold rotate_half's negation into sin
        snT = snT * np.where(neg[:, None], -1.0, 1.0).astype(np.float32)
        # prearrange to the kernel's SBUF layouts for contiguous DMA runs:
        #   x16: [p, st, c, s] = xT[c*128+p, st*512+s]
        #   w*:  [p, c, d]     = wT[c*128+p, d]
        #   wo:  [p, h, e]     = woT[h*128+p, e]
        x_p = np.ascontiguousarray(
            xT.reshape(NHC, 128, S // 512, 512).transpose(1, 2, 0, 3)
        )
        def wlay(wT):  # [HID, DH] -> [128, NHC, DH]
            return np.ascontiguousarray(wT.reshape(NHC, 128, DH).transpose(1, 0, 2))
        woT = np.ascontiguousarray(wo[:, sl].T)  # [DH, HID]
        wo_p = np.ascontiguousarray(woT.reshape(HPC, 128, HID).transpose(1, 0, 2))
        in_maps.append(
            {
                "x16": x_p.astype(bf),
                "wq16": wlay(wq_sl.T).astype(bf),
                "wk16": wlay(wk_sl.T).astype(bf),
                "wv16": wlay(wv[sl].T).astype(bf),
                "wo16": wo_p.astype(bf),
                "cs16": np.ascontiguousarray(csT).astype(bf),
                "sn16": np.ascontiguousarray(snT).astype(bf),
            }
        )
    return in_maps


def kernel(
    hidden_states,
    cos,
    sin,
    wq,
    wk,
    wv,
    wo,
    position_ids=None,
    _trace=False,
    _tmpdir=None,
):
    global LAST_EXEC_TIME_NS
    if "nc" not in _CACHE:
        _CACHE["nc"] = _build_device_program()
    nc = _CACHE["nc"]
    in_maps = _host_inputs(hidden_states, cos, sin, wq, wk, wv, wo)
    res = run_bass_kernel_spmd(
        nc,
        in_maps,
        list(range(NCORES)),
        trace=_trace,
        tmpdir=_tmpdir,
    )
    LAST_EXEC_TIME_NS = res.exec_time_ns
    full = np.zeros((B, S, HID), dtype=np.float32)
    for c in range(NCORES):
        full[c // BGROUP] += res.results[c]["out"].astype(np.float32)
    return full
